# revision 1
# baseline (speedup 1.0000x reference)
"""Masked phase-locking value (PLV) kernel for Trainium2, 8 NeuronCores.

Math: out[b] = |sum_ij M_ij * exp(i*(a_bi - b_bj))| / max(sum(M), 1)
    real_b = sum_ij M_ij (cos a_bi cos b_bj + sin a_bi sin b_bj)
    imag_b = sum_ij M_ij (sin a_bi cos b_bj - cos a_bi sin b_bj)

Device decomposition (per core, Na sharded 8 ways -> 1024 rows each):
    acc[m, j] = sum_i W[i, m] * mask[i, j]     (TensorE; W = [ca^T | sa^T], m = 2B = 128)
    racc[m]   = sum_j acc[m, j] * CS[m, j]     (DVE mult, ACT accumulate; CS = [cb; sb])
    qacc[m]   = sum_j acc[m, j] * SW[m, j]     (SW = [sb; cb], partition-swap of CS)
real_b = sum_cores racc[b] + racc[64+b]; imag_b = sum_cores qacc[64+b] - qacc[b].
All bilinear in mask rows, so Na-shard partials just add; host does the tiny
fold + |z| / sum(M).

dtypes: mask is 0/1 -> exact in fp8e4 (1 byte, halves HBM traffic, full PE rate);
weights/CS fp16 (PE full rate); SW in fp8 (the imag side is an incoherent sum,
tiny vs the coherent real part, so fp8 there costs ~1e-5 extra error);
PSUM/epilogue fp32. End-to-end rel err ~2e-5.
Column groups are sized small-big-small: a small first group starts the PE
early, small last groups shorten the end-of-stream epilogue tail. Trig rides
the scalar HWDGE ring (doesn't queue behind masks); a PE warm-up burst during
the DMA lead-in defeats the HAM cold-clock penalty.
"""

import numpy as np

import concourse.bass as bass
import concourse.tile as tile
from concourse import bacc, mybir
from concourse.bass_utils import run_bass_kernel_spmd

B = 64
NA = 8192
NB = 8192
NCORES = 8
NASH = NA // NCORES          # mask rows per core
KCH = NASH // 128            # contraction chunks of 128 rows
NCH = 512                    # output columns per PSUM bank / matmul

# column group widths: small first (early PE start), small last (short tail)
GWS = [512, 1024, 1024, 1024, 1024, 1024, 1024, 512, 512, 256, 256]
assert sum(GWS) == NB and all(w % 256 == 0 for w in GWS)
NG = len(GWS)
GOFF = [sum(GWS[:i]) for i in range(NG)]

# trig upload pieces (scalar ring): first small so group 0's epilogue isn't gated
TP = [1024, 2048, 2560, 2560]
assert sum(TP) == NB
TPOFF = [sum(TP[:i]) for i in range(len(TP))]

F8 = mybir.dt.float8e4
F16 = mybir.dt.float16
F32 = mybir.dt.float32


def build_program() -> bass.Bass:
    nc = bacc.Bacc("TRN2")
    # concatenated per-group blocks, each contiguous [128, KCH, gw]
    mask_d = nc.dram_tensor("mask", [128 * KCH * NB], F8, kind="ExternalInput")
    w_d = nc.dram_tensor("w", [128, KCH, 2 * B], F16, kind="ExternalInput")
    cs_d = nc.dram_tensor("cs", [128, NB], F16, kind="ExternalInput")
    sw_d = nc.dram_tensor("sw", [128, NB], F8, kind="ExternalInput")
    out_d = nc.dram_tensor("out", [128, 2 * NG], F32, kind="ExternalOutput")

    copy_f = mybir.ActivationFunctionType.Copy

    with tile.TileContext(nc) as tc:
        with (
            tc.tile_pool(name="consts", bufs=1) as consts,
            tc.tile_pool(name="masks", bufs=NG) as masks,
            tc.tile_pool(name="scratch", bufs=3) as scratch,
            tc.tile_pool(name="junk", bufs=2) as junkp,
            tc.tile_pool(name="psum", bufs=3, space="PSUM") as psum_pool,
            tc.tile_pool(name="wups", bufs=1, space="PSUM") as wu_pool,
        ):
            w_sb = consts.tile([128, KCH, 2 * B], F16)
            nc.sync.dma_start(out=w_sb[:], in_=w_d[:])
            cs_sb = consts.tile([128, NB], F16)
            sw_sb = consts.tile([128, NB], F8)
            racc = consts.tile([128, 2 * NG], F32)

            # PE warm-up while the first mask group is in flight (HAM ramp)
            wu_ps = wu_pool.tile([128, 2 * B], F32)
            for r in range(16):
                nc.tensor.matmul(
                    out=wu_ps[:],
                    lhsT=w_sb[:, 0, :],
                    rhs=w_sb[:, 1, :],
                    start=(r == 0),
                    stop=(r == 15),
                )

            tp_emitted = 0
            for g in range(NG):
                off, gw = GOFF[g], GWS[g]
                gsl = slice(off, off + gw)
                mt = masks.tile([128, KCH, gw], F8, tag="mask")
                blk = 128 * KCH
                src = mask_d[off * blk : (off + gw) * blk].rearrange(
                    "(p k c) -> p k c", p=128, k=KCH
                )
                nc.sync.dma_start(out=mt[:], in_=src)
                # trig pieces on the scalar HWDGE ring, paced ahead of use
                while tp_emitted < len(TP) and TPOFF[tp_emitted] < off + gw:
                    tsl = slice(TPOFF[tp_emitted], TPOFF[tp_emitted] + TP[tp_emitted])
                    nc.scalar.dma_start(out=cs_sb[:, tsl], in_=cs_d[:, tsl])
                    nc.scalar.dma_start(out=sw_sb[:, tsl], in_=sw_d[:, tsl])
                    tp_emitted += 1

                ps = psum_pool.tile([128, gw], F32, tag="psum")
                for j0 in range(0, gw, NCH):
                    jsl = slice(j0, min(j0 + NCH, gw))
                    for k in range(KCH):
                        nc.tensor.matmul(
                            out=ps[:, jsl],
                            lhsT=w_sb[:, k, :],
                            rhs=mt[:, k, jsl],
                            start=(k == 0),
                            stop=(k == KCH - 1),
                        )
                rcol = g if g < 8 else 16 + (g - 8)
                qcol = 8 + g if g < 8 else 16 + (NG - 8) + (g - 8)
                pr = scratch.tile([128, gw], F32, tag="pr")
                nc.vector.tensor_mul(out=pr[:], in0=ps[:], in1=cs_sb[:, gsl])
                jr = junkp.tile([128, gw], F32, tag="junk")
                nc.scalar.activation(
                    out=jr[:], in_=pr[:], func=copy_f,
                    accum_out=racc[:, rcol : rcol + 1],
                )
                pi = scratch.tile([128, gw], F32, tag="pr")
                nc.vector.tensor_mul(out=pi[:], in0=ps[:], in1=sw_sb[:, gsl])
                ji = junkp.tile([128, gw], F32, tag="junk")
                nc.scalar.activation(
                    out=ji[:], in_=pi[:], func=copy_f,
                    accum_out=racc[:, qcol : qcol + 1],
                )
                if g == 7:
                    # groups 0-7 partials fly out while tail groups finish
                    nc.sync.dma_start(out=out_d[:, :16], in_=racc[:, :16])

            nc.sync.dma_start(out=out_d[:, 16:], in_=racc[:, 16:])
    nc.finalize()
    return nc


def prep_inputs(phases_a, phases_b, coupling_mask):
    pa = np.asarray(phases_a, dtype=np.float32)
    pb = np.asarray(phases_b, dtype=np.float32)
    ca, sa = np.cos(pa), np.sin(pa)
    cb, sb = np.cos(pb), np.sin(pb)
    cs = np.concatenate([cb, sb], axis=0).astype(np.float16)
    sw = np.concatenate([sb, cb], axis=0).astype(mybir.dt.np(F8))

    f8np = mybir.dt.np(F8)
    one_byte = np.array([1.0], f8np).view(np.uint8)[0]
    mask_u8 = (np.asarray(coupling_mask) != 0).astype(np.uint8) * one_byte

    in_maps = []
    for c in range(NCORES):
        rows = slice(c * NASH, (c + 1) * NASH)
        W = np.empty((NASH, 2 * B), np.float16)
        W[:, :B] = ca[:, rows].T
        W[:, B:] = sa[:, rows].T
        # [i = k*128 + p, m] -> [p, k, m]
        w_host = np.ascontiguousarray(W.reshape(KCH, 128, 2 * B).transpose(1, 0, 2))
        # per group: contiguous [p, k, c] block; blocks concatenated flat
        mr = mask_u8[rows].reshape(KCH, 128, NB)
        blocks = [
            np.ascontiguousarray(
                mr[:, :, GOFF[g] : GOFF[g] + GWS[g]].transpose(1, 0, 2)
            ).reshape(-1)
            for g in range(NG)
        ]
        m_host = np.concatenate(blocks).view(f8np)
        in_maps.append({"mask": m_host, "w": w_host, "cs": cs, "sw": sw})
    return in_maps


def combine(outs, coupling_mask):
    o = np.stack(outs).astype(np.float64)  # [NCORES, 128, 2*NG]
    nt = NG - 8
    r = o[:, :, :8].sum(axis=2) + o[:, :, 16 : 16 + nt].sum(axis=2)
    q = o[:, :, 8:16].sum(axis=2) + o[:, :, 16 + nt :].sum(axis=2)
    real = (r[:, :B] + r[:, B:]).sum(axis=0)
    imag = (q[:, B:] - q[:, :B]).sum(axis=0)
    n_pairs = max(float(np.asarray(coupling_mask).sum()), 1.0)
    return (np.sqrt(real * real + imag * imag) / n_pairs).astype(np.float32)


_prog_cache: list = []


def kernel(phases_a, phases_b, coupling_mask):
    in_maps = prep_inputs(phases_a, phases_b, coupling_mask)
    if not _prog_cache:
        _prog_cache.append(build_program())
    res = run_bass_kernel_spmd(_prog_cache[0], in_maps, core_ids=list(range(NCORES)))
    return combine([r["out"] for r in res.results], coupling_mask)



# revision 4
# speedup vs baseline: 1.2313x; 1.2313x over previous
"""Masked phase-locking value (PLV) kernel for Trainium2, 8 NeuronCores.

Math: out[b] = |sum_ij M_ij * exp(i*(a_bi - b_bj))| / max(sum(M), 1)
    real_b = ca_b^T M cb_b + sa_b^T M sb_b
    imag_b = sa_b^T M cb_b - ca_b^T M sb_b

Device decomposition (per core, Na sharded 8 ways -> 1024 rows each),
*transposed* orientation so the j-contraction (Nb = 8192) runs on the PE:

    Z[m, i] = sum_j CS[j, m] * maskT[j, i]      (TensorE; CS = [cb^T | sb^T],
                                                 m = 2B = 128, i = 1024)
    racc[m] = sum_i Z[m, i] * WR[m, i]          (one DVE tensor_tensor_reduce)
    qacc[m] = sum_i Z[m, i] * WI[m, i]          (WR = [ca|sa], WI = [sa|-ca])

real_b = sum_cores racc[b] + racc[64+b]; imag_b = qacc[b] + qacc[64+b].
vs the old orientation, this shrinks the epilogue 8x (the old scheme reduced
over j = 8192 on DVE+ACT; here the PE does that, leaving [128, 1024]).

The j-accumulation is split in two PSUM tiles (jc 0-31 -> Za, 32-63 -> Zb) so
the first epilogue half overlaps the second half's matmuls; only ~2 DVE ops
(~1 us) trail the last matmul.

dtypes: mask 0/1 in fp8e4 (exact, 1 byte); CS fp16 stationary; epilogue fp32.
Mask rides the sync HWDGE ring in large (0.25-1 MB) chunks; CS/WR ride the
scalar ring, paced ahead of PE consumption. A PE warm-up burst during the DMA
lead-in defeats the HAM cold-clock penalty.
"""

import numpy as np

import concourse.bass as bass
import concourse.tile as tile
from concourse import bacc, mybir
from concourse.bass_utils import run_bass_kernel_spmd

B = 64
NA = 8192
NB = 8192
NCORES = 8
NISH = NA // NCORES          # mask rows (i) per core
JCH = NB // 128              # j contraction chunks of 128

# mask DMA groups in jc units: small first (early PE start), then big
GJ = [2, 2, 4, 8, 8, 8, 8, 8, 8, 8]
assert sum(GJ) == JCH
GOFF = [sum(GJ[:i]) for i in range(len(GJ))]

# cs upload pieces (scalar ring), in jc units, paced ahead of use
CSP = [4, 12, 16, 16, 16]
assert sum(CSP) == JCH
CSOFF = [sum(CSP[:i]) for i in range(len(CSP))]

F8 = mybir.dt.float8e4
F16 = mybir.dt.float16
F32 = mybir.dt.float32

HALF = JCH // 2              # jc < HALF -> Za, else Zb


def build_program() -> bass.Bass:
    nc = bacc.Bacc("TRN2")
    mask_d = nc.dram_tensor("mask", [128, JCH, NISH], F8, kind="ExternalInput")
    cs_d = nc.dram_tensor("cs", [128, JCH, 2 * B], F16, kind="ExternalInput")
    wr_d = nc.dram_tensor("wr", [128, 2, NISH], F16, kind="ExternalInput")
    out_d = nc.dram_tensor("out", [128, 4], F32, kind="ExternalOutput")

    mul = mybir.AluOpType.mult
    add = mybir.AluOpType.add

    with tile.TileContext(nc) as tc:
        with (
            tc.tile_pool(name="consts", bufs=1) as consts,
            tc.tile_pool(name="masks", bufs=len(GJ)) as masks,
            tc.tile_pool(name="junk", bufs=2) as junkp,
            tc.tile_pool(name="psum", bufs=1, space="PSUM") as psum_pool,
            tc.tile_pool(name="wups", bufs=1, space="PSUM") as wu_pool,
        ):
            wr_sb = consts.tile([128, 2, NISH], F16)
            nc.scalar.dma_start(out=wr_sb[:], in_=wr_d[:])
            cs_sb = consts.tile([128, JCH, 2 * B], F16)
            nc.scalar.dma_start(
                out=cs_sb[:, 0 : CSP[0], :], in_=cs_d[:, 0 : CSP[0], :]
            )
            racc = consts.tile([128, 4], F32)

            # PE warm-up while the first mask groups are in flight (HAM ramp)
            wu_ps = wu_pool.tile([128, 512], F32)
            for r in range(8):
                nc.tensor.matmul(
                    out=wu_ps[:],
                    lhsT=cs_sb[:, 0, :],
                    rhs=wr_sb[:, 0, 0:512],
                    start=(r == 0),
                    stop=(r == 7),
                )

            za = psum_pool.tile([128, NISH], F32, tag="za")
            zb = psum_pool.tile([128, NISH], F32, tag="zb")
            zt = [za, zb]

            cs_emitted = 1
            for g, gj in enumerate(GJ):
                jc0 = GOFF[g]
                mt = masks.tile([128, gj, NISH], F8, tag="mask")
                nc.sync.dma_start(out=mt[:], in_=mask_d[:, jc0 : jc0 + gj, :])
                # pace cs pieces ahead of consumption
                while cs_emitted < len(CSP) and CSOFF[cs_emitted] < jc0 + gj:
                    p0, pw = CSOFF[cs_emitted], CSP[cs_emitted]
                    nc.scalar.dma_start(
                        out=cs_sb[:, p0 : p0 + pw, :], in_=cs_d[:, p0 : p0 + pw, :]
                    )
                    cs_emitted += 1

                for k in range(gj):
                    jc = jc0 + k
                    z = zt[jc // HALF]
                    jl = jc % HALF
                    for i0 in range(0, NISH, 512):
                        nc.tensor.matmul(
                            out=z[:, i0 : i0 + 512],
                            lhsT=cs_sb[:, jc, :],
                            rhs=mt[:, k, i0 : i0 + 512],
                            start=(jl == 0),
                            stop=(jl == HALF - 1),
                        )

                if jc0 + gj == HALF:
                    # Za complete: first epilogue half overlaps Zb matmuls
                    for q in range(2):
                        jr = junkp.tile([128, NISH], F16, tag="junk")
                        nc.vector.scalar_tensor_tensor(
                            out=jr[:], in0=za[:], scalar=1.0,
                            in1=wr_sb[:, q, :], op0=mul, op1=mul,
                            accum_out=racc[:, q : q + 1],
                        )
                    nc.sync.dma_start(out=out_d[:, 0:2], in_=racc[:, 0:2])

            for q in range(2):
                jr = junkp.tile([128, NISH], F16, tag="junk")
                nc.vector.scalar_tensor_tensor(
                    out=jr[:], in0=zb[:], scalar=1.0,
                    in1=wr_sb[:, q, :], op0=mul, op1=mul,
                    accum_out=racc[:, 2 + q : 3 + q],
                )
            nc.sync.dma_start(out=out_d[:, 2:4], in_=racc[:, 2:4])
    nc.finalize()
    return nc


def prep_inputs(phases_a, phases_b, coupling_mask):
    pa = np.asarray(phases_a, dtype=np.float32)
    pb = np.asarray(phases_b, dtype=np.float32)
    ca, sa = np.cos(pa), np.sin(pa)   # (B, NA)
    cb, sb = np.cos(pb), np.sin(pb)   # (B, NB)

    f8np = mybir.dt.np(F8)
    one_byte = np.array([1.0], f8np).view(np.uint8)[0]
    mask_u8 = (np.asarray(coupling_mask) != 0).astype(np.uint8) * one_byte

    # cs[p, jc, m] = (cb|sb)[m, 128*jc + p] — shared by all cores
    csf = np.concatenate([cb, sb], axis=0).astype(np.float16)    # (128, NB)
    cs_host = np.ascontiguousarray(
        csf.T.reshape(JCH, 128, 2 * B).transpose(1, 0, 2)
    )

    in_maps = []
    for c in range(NCORES):
        rows = slice(c * NISH, (c + 1) * NISH)
        # mask[p, jc, i] = M[rows[i], 128*jc + p]
        mt = np.ascontiguousarray(
            mask_u8[rows].T.reshape(JCH, 128, NISH).transpose(1, 0, 2)
        ).view(f8np)
        wr = np.empty((128, 2, NISH), np.float16)
        wr[:B, 0] = ca[:, rows]
        wr[B:, 0] = sa[:, rows]
        wr[:B, 1] = sa[:, rows]
        wr[B:, 1] = -ca[:, rows]
        in_maps.append({"mask": mt, "cs": cs_host, "wr": wr})
    return in_maps


def combine(outs, coupling_mask):
    o = np.stack(outs).astype(np.float64)          # [NCORES, 128, 4]
    r = (o[:, :, 0] + o[:, :, 2]).sum(axis=0)      # [128]
    q = (o[:, :, 1] + o[:, :, 3]).sum(axis=0)
    real = r[:B] + r[B:]
    imag = q[:B] + q[B:]
    n_pairs = max(float(np.asarray(coupling_mask).sum()), 1.0)
    return (np.sqrt(real * real + imag * imag) / n_pairs).astype(np.float32)


_prog_cache: list = []


def kernel(phases_a, phases_b, coupling_mask):
    in_maps = prep_inputs(phases_a, phases_b, coupling_mask)
    if not _prog_cache:
        _prog_cache.append(build_program())
    res = run_bass_kernel_spmd(_prog_cache[0], in_maps, core_ids=list(range(NCORES)))
    return combine([r["out"] for r in res.results], coupling_mask)


# revision 7
# speedup vs baseline: 1.2392x; 1.0064x over previous
"""Masked phase-locking value (PLV) kernel for Trainium2, 8 NeuronCores.

Math: out[b] = |sum_ij M_ij * exp(i*(a_bi - b_bj))| / max(sum(M), 1)
    real_b = ca_b^T M cb_b + sa_b^T M sb_b
    imag_b = sa_b^T M cb_b - ca_b^T M sb_b

Device decomposition (per core, Na sharded 8 ways -> 1024 rows each),
*transposed* orientation so the j-contraction (Nb = 8192) runs on the PE:

    Z[m, i] = sum_j CS[j, m] * maskT[j, i]      (TensorE; CS = [cb^T | sb^T],
                                                 m = 2B = 128, i = 1024)
    racc[m] = sum_i Z[m, i] * WR[m, i]          (DVE scalar_tensor_tensor)
    qacc[m] = sum_i Z[m, i] * WI[m, i]          (GpSimd; WR = [ca|sa], WI = [sa|-ca])

real_b = sum_cores racc[b] + racc[64+b]; imag_b = qacc[b] + qacc[64+b].
vs the j-reduce-on-DVE orientation this shrinks the epilogue 8x; the PE does
the big reduction.

The j-accumulation is split in two PSUM tiles (jc 0-31 -> Za, 32-63 -> Zb) so
the first epilogue half overlaps the second half's matmuls.

DMA: mask groups alternate between the two HWDGE rings (sync + scalar) so the
per-ring dma_start kick (~0.6 us DIRECT2D) and inter-group gaps overlap
between rings; cs (fp8) pieces lead each ring. PE warm-up runs from a memset
tile so it needs no DMA and beats the HAM cold clock during the DMA lead-in.
dtypes: mask 0/1 fp8e4 (exact); cs fp8e4 (b-side quantization noise is
incoherent, ~3e-4 of the coherent real part); wr fp16; PSUM/epilogue fp32.
"""

import numpy as np

import concourse.bass as bass
import concourse.tile as tile
from concourse import bacc, mybir
from concourse.bass_utils import run_bass_kernel_spmd

B = 64
NA = 8192
NB = 8192
NCORES = 8
NISH = NA // NCORES          # mask rows (i) per core
JCH = NB // 128              # j contraction chunks of 128

# mask DMA groups in jc units; ring alternates per group.
# A boundary must land exactly at HALF (32) for the Za epilogue trigger.
GJ = [2, 2, 4, 8, 8, 8, 10, 10, 12]
assert sum(GJ) == JCH
assert 32 in [sum(GJ[: i + 1]) for i in range(len(GJ))]
GOFF = [sum(GJ[:i]) for i in range(len(GJ))]

# cs upload pieces, alternating rings ahead of the mask groups
CSP = [4, 12, 16, 32]
assert sum(CSP) == JCH
CSOFF = [sum(CSP[:i]) for i in range(len(CSP))]

F8 = mybir.dt.float8e4
F16 = mybir.dt.float16
F32 = mybir.dt.float32

HALF = JCH // 2              # jc < HALF -> Za, else Zb


def build_program() -> bass.Bass:
    nc = bacc.Bacc("TRN2")
    mask_d = nc.dram_tensor("mask", [128, JCH, NISH], F8, kind="ExternalInput")
    cs_d = nc.dram_tensor("cs", [128, JCH, 2 * B], F8, kind="ExternalInput")
    wr_d = nc.dram_tensor("wr", [128, 2, NISH], F16, kind="ExternalInput")
    out_d = nc.dram_tensor("out", [128, 4], F32, kind="ExternalOutput")

    mul = mybir.AluOpType.mult
    rings = [nc.sync, nc.scalar]

    with tile.TileContext(nc) as tc:
        with (
            tc.tile_pool(name="consts", bufs=1) as consts,
            tc.tile_pool(name="masks", bufs=len(GJ)) as masks,
            tc.tile_pool(name="junk", bufs=2) as junkp,
            tc.tile_pool(name="psum", bufs=1, space="PSUM") as psum_pool,
            tc.tile_pool(name="wups", bufs=1, space="PSUM") as wu_pool,
        ):
            # engine-local warm-up operand: no DMA dependency
            wu_sb = consts.tile([128, 512], F16)
            nc.vector.memset(wu_sb[:], 0.0)

            cs_sb = consts.tile([128, JCH, 2 * B], F8)
            rings[0].dma_start(out=cs_sb[:, 0 : CSP[0], :], in_=cs_d[:, 0 : CSP[0], :])
            rings[1].dma_start(
                out=cs_sb[:, CSP[0] : CSOFF[2], :], in_=cs_d[:, CSP[0] : CSOFF[2], :]
            )
            wr_sb = consts.tile([128, 2, NISH], F16)
            racc = consts.tile([128, 4], F32)

            # PE warm-up while the first mask groups are in flight (HAM ramp)
            wu_ps = wu_pool.tile([128, 512], F32)
            for r in range(10):
                nc.tensor.matmul(
                    out=wu_ps[:],
                    lhsT=wu_sb[:, 0:128],
                    rhs=wu_sb[:],
                    start=(r == 0),
                    stop=(r == 9),
                )

            za = psum_pool.tile([128, NISH], F32, tag="za")
            zb = psum_pool.tile([128, NISH], F32, tag="zb")
            zt = [za, zb]

            cs_emitted = 2
            wr_emitted = False
            for g, gj in enumerate(GJ):
                jc0 = GOFF[g]
                ring = rings[g % 2]
                mt = masks.tile([128, gj, NISH], F8, tag="mask")
                ring.dma_start(out=mt[:], in_=mask_d[:, jc0 : jc0 + gj, :])
                if not wr_emitted and jc0 + gj >= 8:
                    # wr needed first by the za epilogue (after jc 31)
                    rings[(g + 1) % 2].dma_start(out=wr_sb[:], in_=wr_d[:])
                    wr_emitted = True
                while cs_emitted < len(CSP) and CSOFF[cs_emitted] < jc0 + gj + 8:
                    p0, pw = CSOFF[cs_emitted], CSP[cs_emitted]
                    rings[(g + 1) % 2].dma_start(
                        out=cs_sb[:, p0 : p0 + pw, :], in_=cs_d[:, p0 : p0 + pw, :]
                    )
                    cs_emitted += 1

                for k in range(gj):
                    jc = jc0 + k
                    z = zt[jc // HALF]
                    jl = jc % HALF
                    for i0 in range(0, NISH, 512):
                        nc.tensor.matmul(
                            out=z[:, i0 : i0 + 512],
                            lhsT=cs_sb[:, jc, :],
                            rhs=mt[:, k, i0 : i0 + 512],
                            start=(jl == 0),
                            stop=(jl == HALF - 1),
                        )

                if jc0 + gj == HALF:
                    # Za complete: epilogue overlaps Zb matmuls (DVE + Pool)
                    for q, eng in ((0, nc.vector), (1, nc.vector)):
                        jr = junkp.tile([128, NISH], F16, tag="junk")
                        eng.scalar_tensor_tensor(
                            out=jr[:], in0=za[:], scalar=1.0,
                            in1=wr_sb[:, q, :], op0=mul, op1=mul,
                            accum_out=racc[:, q : q + 1],
                        )
                    nc.sync.dma_start(out=out_d[:, 0:2], in_=racc[:, 0:2])

            for q, eng in ((0, nc.vector), (1, nc.vector)):
                jr = junkp.tile([128, NISH], F16, tag="junk")
                eng.scalar_tensor_tensor(
                    out=jr[:], in0=zb[:], scalar=1.0,
                    in1=wr_sb[:, q, :], op0=mul, op1=mul,
                    accum_out=racc[:, 2 + q : 3 + q],
                )
            nc.scalar.dma_start(out=out_d[:, 2:4], in_=racc[:, 2:4])
    nc.finalize()
    return nc


def prep_inputs(phases_a, phases_b, coupling_mask):
    pa = np.asarray(phases_a, dtype=np.float32)
    pb = np.asarray(phases_b, dtype=np.float32)
    ca, sa = np.cos(pa), np.sin(pa)   # (B, NA)
    cb, sb = np.cos(pb), np.sin(pb)   # (B, NB)

    f8np = mybir.dt.np(F8)
    one_byte = np.array([1.0], f8np).view(np.uint8)[0]
    mask_u8 = (np.asarray(coupling_mask) != 0).astype(np.uint8) * one_byte

    # cs[p, jc, m] = (cb|sb)[m, 128*jc + p] — shared by all cores
    csf = np.concatenate([cb, sb], axis=0).astype(f8np)          # (128, NB)
    cs_host = np.ascontiguousarray(
        csf.T.reshape(JCH, 128, 2 * B).transpose(1, 0, 2)
    )

    in_maps = []
    for c in range(NCORES):
        rows = slice(c * NISH, (c + 1) * NISH)
        # mask[p, jc, i] = M[rows[i], 128*jc + p]
        mt = np.ascontiguousarray(
            mask_u8[rows].T.reshape(JCH, 128, NISH).transpose(1, 0, 2)
        ).view(f8np)
        wr = np.empty((128, 2, NISH), np.float16)
        wr[:B, 0] = ca[:, rows]
        wr[B:, 0] = sa[:, rows]
        wr[:B, 1] = sa[:, rows]
        wr[B:, 1] = -ca[:, rows]
        in_maps.append({"mask": mt, "cs": cs_host, "wr": wr})
    return in_maps


def combine(outs, coupling_mask):
    o = np.stack(outs).astype(np.float64)          # [NCORES, 128, 4]
    r = (o[:, :, 0] + o[:, :, 2]).sum(axis=0)      # [128]
    q = (o[:, :, 1] + o[:, :, 3]).sum(axis=0)
    real = r[:B] + r[B:]
    imag = q[:B] + q[B:]
    n_pairs = max(float(np.asarray(coupling_mask).sum()), 1.0)
    return (np.sqrt(real * real + imag * imag) / n_pairs).astype(np.float32)


_prog_cache: list = []


def kernel(phases_a, phases_b, coupling_mask):
    in_maps = prep_inputs(phases_a, phases_b, coupling_mask)
    if not _prog_cache:
        _prog_cache.append(build_program())
    res = run_bass_kernel_spmd(_prog_cache[0], in_maps, core_ids=list(range(NCORES)))
    return combine([r["out"] for r in res.results], coupling_mask)
